# revision 31
# baseline (speedup 1.0000x reference)
"""Trainium2 Bass kernel for nn_DecoderA (neural BP / GNN message passing decoder).

Strategy: pure data parallel over batch (128 items -> 8 cores x 16 items).
Per core, items run in 4 groups of 4 (two groups interleaved per iteration
for cross-engine overlap); each group's message state M [4*288, 576] lives
in SBUF as 9 tiles of [128, 576] ((b,m)-rows x n).  Per iteration, per tile:

  PE    psumA = Esel@A            (fp32r matmul, 1 cyc/row; A = x_t + post)
  DVE   v     = psumA - M         (raw V; off-edge lanes carry garbage)
  ACT   te    = tanh(0.5 v)       (pair-wide)
  DVE   te    = clip(te, +-tanh(7.5))  (== reference's clip(V, +-15))
  DVE   P     = reduce_mult(max(te, offm))  (offm: +1 off-edge / -1 on-edge;
                                   fixes off-edge factors to +1 in the product)
  ACT   a1,a2 = |te +- P|         (Abs with bias=+-P) -> bf16
  ACT   l     = ln(a + 1e-30)     (guard only for te == P == 0) -> fp16 view
  GPS   lq    = l1 - l2           (== 2*atanh(P/te), division-free)
  DVE   lqc   = clip(lq, +-14.5)
  GPS   d     = lqc * wg          (wg = gate*w_cv*H fp16; zero off-edge)
  DVE   M     = (1-gate)*M + d    (damped update; t=0: M = d)
  PE    post += Esel^T @ d        (fp16 matmul; posts(t) = (1-gate)*posts(t-1)
                                   + sum_m d recurrence replaces sum_m M)

Off-edge lanes are never masked: wg=0 kills them in d, and offm fixes them
inside the product reduce.  The log-magnitude tail runs in fp16 (bitcast
views of the bf16 staging bytes) because ln outputs are ~+-14 and bf16's
0.4% relative error there breaks the 2e-2 output gate.
Host does the cheap pre/post work (LLR normalization, pooling, sigmoid).
"""

import sys

import numpy as np

sys.path.insert(0, "/opt/trn_rl_repo")

import ml_dtypes  # noqa: E402

import concourse.bacc as bacc  # noqa: E402
import concourse.tile as tile  # noqa: E402
from concourse import mybir  # noqa: E402
from concourse.bass_utils import run_bass_kernel_spmd  # noqa: E402

F32 = mybir.dt.float32
F32R = mybir.dt.float32r
BF16 = mybir.dt.bfloat16
F16 = mybir.dt.float16


def _f16(ap):
    return ap.bitcast(mybir.dt.float16)
ALU = mybir.AluOpType
ACT = mybir.ActivationFunctionType

B = 128
MCHK = 288
NVAR = 576
KINFO = 288
T = 5
NCORES = 8
BL = B // NCORES          # 16 items per core
GI = 4                    # items per group
NG = BL // GI             # 4 groups
NT = GI * MCHK // 128     # 9 tiles of [128, NVAR] per group
HC = NVAR // 2            # 288, matmul N-chunk (<=512 per PSUM bank)

_GUARD = 1e-30            # ln guard: avoids ln(0) when te == P == 0
_CLIP_C = float(2.0 * np.arctanh(np.float64(np.float32(1.0 - 1e-6))))
_TE_CAP = float(np.float32(np.tanh(np.float64(np.float32(7.5)))))


def _build(gate: float):
    nc = bacc.Bacc("TRN2", target_bir_lowering=False, debug=False)

    wg_d = nc.dram_tensor("wg", [BL * MCHK, NVAR], F16, kind="ExternalInput").ap()
    offm_d = nc.dram_tensor("offm", [BL * MCHK, NVAR], BF16,
                            kind="ExternalInput").ap()
    xs_d = nc.dram_tensor("xs", [BL, T * NVAR], F32, kind="ExternalInput").ap()
    esel_d = nc.dram_tensor("esel", [128, NT * GI], F16, kind="ExternalInput").ap()
    eselt_d = nc.dram_tensor("eselt", [GI, NT * 128], F32R,
                             kind="ExternalInput").ap()
    posts_d = nc.dram_tensor("posts", [BL, T * NVAR], F32, kind="ExternalOutput").ap()

    one_m_g = float(1.0 - gate)

    with tile.TileContext(nc) as tc:
        with (
            tc.tile_pool(name="consts", bufs=1) as consts,
            tc.tile_pool(name="wg", bufs=2) as wg_pool,
            tc.tile_pool(name="offm", bufs=2) as offm_pool,
            tc.tile_pool(name="mstate", bufs=2) as m_pool,
            tc.tile_pool(name="atile", bufs=3) as a_pool,
            tc.tile_pool(name="vte", bufs=3) as vte_pool,
            tc.tile_pool(name="a12", bufs=2) as a12_pool,
            tc.tile_pool(name="pprod", bufs=2) as p_pool,
            tc.tile_pool(name="psum_v", bufs=2, space="PSUM") as psv_pool,
            tc.tile_pool(name="psum_post", bufs=2, space="PSUM") as psp_pool,
        ):
            esel = consts.tile([128, NT, GI], F16)
            nc.sync.dma_start(out=esel, in_=esel_d.rearrange("p (j g) -> p j g", g=GI))
            eselt = consts.tile([GI, NT, 128], F32R)
            nc.sync.dma_start(
                out=eselt, in_=eselt_d.rearrange("g (j p) -> g j p", p=128)
            )
            b_guard = consts.tile([128, 1], F32)
            nc.vector.memset(b_guard, _GUARD)
            xsall = consts.tile([128, T, 2, HC], F32)
            for g in range(NG):
                nc.sync.dma_start(
                    out=xsall[32 * g : 32 * g + GI],
                    in_=xs_d[g * GI : (g + 1) * GI].rearrange(
                        "b (t c n) -> b t c n", t=T, c=2
                    ),
                )
            postsall = consts.tile([128, T, 2, HC], F32)

            # tile pairs: (0,1) (2,3) (4,5) (6,7) (8,)
            pairs = [(0, 1), (2, 3), (4, 5), (6, 7), (8,)]

            def load_group(g):
                wg_g = wg_pool.tile([128, NT, NVAR], F16)
                nc.sync.dma_start(
                    out=wg_g,
                    in_=wg_d[g * NT * 128 : (g + 1) * NT * 128, :].rearrange(
                        "(j p) n -> p j n", p=128
                    ),
                )
                offm_g = offm_pool.tile([128, NT, NVAR], BF16)
                nc.sync.dma_start(
                    out=offm_g,
                    in_=offm_d[g * NT * 128 : (g + 1) * NT * 128, :].rearrange(
                        "(j p) n -> p j n", p=128
                    ),
                )
                m_g = m_pool.tile([128, NT, NVAR], F32)
                a_cur = a_pool.tile([GI, 2, HC], F32R, tag="a_cur", name="a_cur")
                nc.vector.tensor_copy(a_cur, xsall[32 * g : 32 * g + GI, 0])
                return {"wg": wg_g, "offm": offm_g, "m": m_g, "a": a_cur}

            def stage_a(g, t, st):
                a_cur = st["a"]
                m_g = st["m"]
                st["post_ps"] = psp_pool.tile([GI, 2, 512], F32, tag="post_ps",
                                              name="post_ps")
                st["ptile"] = p_pool.tile([128, NT], F32, tag="pp", name="pp")
                st["a12"] = a12_pool.tile([128, 2, NT, NVAR], BF16,
                                          tag="a12", name="a12")
                st["vtes"] = {}
                for pi, pj in enumerate(pairs):
                    w = len(pj)
                    vte = vte_pool.tile([128, 2, NVAR], F32, tag="vte",
                                        name="vte")[:, :w]
                    st["vtes"][pi] = vte
                    for jj, j in enumerate(pj):
                        v_ps = psv_pool.tile([128, 2, 512], F32)
                        for c in range(2):
                            nc.tensor.matmul(
                                v_ps[:, c, :HC],
                                eselt[:, j],
                                a_cur[:, c],
                                start=True,
                                stop=True,
                            )
                        if t == 0:
                            # M = 0: v is just the broadcast A
                            nc.scalar.copy(
                                vte[:, jj].rearrange("p (c n) -> p c n", c=2),
                                v_ps[:, :, :HC],
                            )
                        else:
                            nc.vector.tensor_tensor(
                                out=vte[:, jj].rearrange("p (c n) -> p c n", c=2),
                                in0=v_ps[:, :, :HC],
                                in1=m_g[:, j].rearrange("p (c n) -> p c n", c=2),
                                op=ALU.subtract,
                            )

            def stage_tanh(g, t, st):
                for pi, pj in enumerate(pairs):
                    nc.scalar.activation(st["vtes"][pi], st["vtes"][pi],
                                         ACT.Tanh, bias=0.0, scale=0.5)

            def stage_cap_prod(g, t, st):
                offm_g = st["offm"]
                ptile = st["ptile"]
                for pi, pj in enumerate(pairs):
                    nc.vector.tensor_scalar(
                        out=st["vtes"][pi], in0=st["vtes"][pi], scalar1=_TE_CAP,
                        scalar2=-_TE_CAP, op0=ALU.min, op1=ALU.max,
                    )
                for pi, pj in enumerate(pairs):
                    w = len(pj)
                    te_p = st["vtes"][pi]
                    nc.vector.tensor_tensor(
                        out=te_p, in0=te_p,
                        in1=offm_g[:, pj[0] : pj[0] + w],
                        op=ALU.max,
                    )
                    for jj, j in enumerate(pj):
                        nc.vector.tensor_reduce(
                            out=ptile[:, j : j + 1],
                            in_=st["vtes"][pi][:, jj],
                            axis=mybir.AxisListType.X,
                            op=ALU.mult,
                        )

            def stage_abs(g, t, st):
                ptile = st["ptile"]
                for pi, pj in enumerate(pairs):
                    for jj, j in enumerate(pj):
                        te = st["vtes"][pi][:, jj]
                        p_t = ptile[:, j : j + 1]
                        nc.scalar.activation(st["a12"][:, 0, j], te,
                                             ACT.Abs, bias=p_t, scale=1.0)
                        nc.scalar.activation(st["a12"][:, 1, j], te,
                                             ACT.Abs, bias=p_t, scale=-1.0)

            def stage_ln(g, t, st):
                nc.scalar.activation(
                    _f16(st["a12"]), st["a12"], ACT.Ln, bias=b_guard,
                )

            def stage_tail(g, t, st):
                m_g, wg_g = st["m"], st["wg"]
                post_ps = st["post_ps"]
                a12 = st["a12"]
                xs_g = xsall[32 * g : 32 * g + GI]
                l1 = _f16(a12[:, 0])
                l2 = _f16(a12[:, 1])
                nc.gpsimd.tensor_tensor(out=l1, in0=l1, in1=l2,
                                        op=ALU.subtract)
                nc.vector.tensor_scalar(
                    out=l1, in0=l1, scalar1=_CLIP_C, scalar2=-_CLIP_C,
                    op0=ALU.min, op1=ALU.max,
                )
                d = _f16(a12[:, 1])
                nc.gpsimd.tensor_tensor(out=d, in0=l1, in1=wg_g,
                                        op=ALU.mult)
                if t == 0:
                    nc.scalar.copy(m_g, d)
                else:
                    nc.vector.scalar_tensor_tensor(
                        out=m_g, in0=m_g, scalar=one_m_g, in1=d,
                        op0=ALU.mult, op1=ALU.add,
                    )
                # posterior increment: post += Esel_j^T @ d_j  (f16)
                for j in range(NT):
                    for c in range(2):
                        nc.tensor.matmul(
                            post_ps[:, c, :HC],
                            esel[:, j],
                            _f16(a12[:, 1, j, c * HC : (c + 1) * HC]),
                            start=(j == 0),
                            stop=(j == NT - 1),
                        )
                # posts(t) = (1-gate)*posts(t-1) + sum_m d  (host adds x_t)
                posts_t = postsall[32 * g : 32 * g + GI, t]
                if t == 0:
                    nc.scalar.copy(posts_t, post_ps[:, :, :HC])
                else:
                    nc.vector.scalar_tensor_tensor(
                        out=posts_t,
                        in0=postsall[32 * g : 32 * g + GI, t - 1],
                        scalar=one_m_g,
                        in1=post_ps[:, :, :HC],
                        op0=ALU.mult,
                        op1=ALU.add,
                    )
                if t + 1 < T:
                    a_new = a_pool.tile([GI, 2, HC], F32R, tag="a_cur",
                                        name="a_cur")
                    nc.vector.tensor_add(a_new, posts_t, xs_g[:, t + 1])
                    st["a"] = a_new

            stages = [stage_a, stage_tanh, stage_cap_prod, stage_abs,
                      stage_ln, stage_tail]
            for gp in range(0, NG, 2):
                sts = {g: load_group(g) for g in (gp, gp + 1)}
                for t in range(T):
                    for stage in stages:
                        for g in (gp, gp + 1):
                            stage(g, t, sts[g])

            for g in range(NG):
                nc.sync.dma_start(
                    out=posts_d[g * GI : (g + 1) * GI].rearrange(
                        "b (t c n) -> b t c n", t=T, c=2
                    ),
                    in_=postsall[32 * g : 32 * g + GI],
                )
    nc.compile()
    return nc


_CACHE = {}


def _get_nc(gate: float):
    key = round(gate, 12)
    if key not in _CACHE:
        _CACHE[key] = _build(gate)
    return _CACHE[key]


def _host_prep(inputs, H, sigma2, input_ponderation, w_cv, gate_logit):
    f32 = np.float32
    gate = float(1.0 / (1.0 + np.exp(-np.float64(gate_logit))))

    llrs = (f32(-4.0) * inputs / sigma2).astype(f32)
    norm_llrs = llrs / np.mean(np.abs(llrs), axis=-1, keepdims=True, dtype=f32)
    xs = (norm_llrs[:, None, :] * input_ponderation[None, :, :]).astype(f32)  # [B,T,N]

    Hf = H.astype(f32)
    wg_full = (f32(gate) * w_cv[None, :, :] * Hf).astype(np.float16)
    offm_full = (f32(1.0) - f32(2.0) * Hf).astype(ml_dtypes.bfloat16)  # +1 off, -1 on

    # selector constants (same for every core)
    rows = np.arange(GI * MCHK)
    esel = np.zeros((128, NT, GI), f32)
    eselt = np.zeros((GI, NT, 128), f32)
    for j in range(NT):
        for p in range(128):
            k = int(rows[j * 128 + p] // MCHK)
            esel[p, j, k] = 1.0
            eselt[k, j, p] = 1.0

    in_maps = []
    for c in range(NCORES):
        sl = slice(c * BL, (c + 1) * BL)
        in_maps.append(
            {
                "wg": np.ascontiguousarray(wg_full[sl].reshape(BL * MCHK, NVAR)),
                "offm": np.ascontiguousarray(offm_full[sl].reshape(BL * MCHK, NVAR)),
                "xs": np.ascontiguousarray(xs[sl].reshape(BL, T * NVAR)),
                "esel": np.ascontiguousarray(
                    esel.reshape(128, NT * GI).astype(np.float16)
                ),
                "eselt": np.ascontiguousarray(eselt.reshape(GI, NT * 128)),
            }
        )
    return gate, norm_llrs, xs, in_maps


def _host_post(posts_raw, xs, norm_llrs, out_ponderation, skip_ponderation):
    f32 = np.float32
    posts = (posts_raw + xs).astype(f32)  # add x_t back in
    norm_out = posts / np.mean(np.abs(posts), axis=-1, keepdims=True, dtype=f32)
    pooled = np.mean(out_ponderation[None] * norm_out, axis=-2, dtype=f32)
    out = (pooled + skip_ponderation * norm_llrs).astype(f32)
    return (1.0 / (1.0 + np.exp(out[:, :KINFO], dtype=f32))).astype(f32)


def run(trace=False, **inputs):
    inputs = {k: np.asarray(v) for k, v in inputs.items()}
    gate, norm_llrs, xs, in_maps = _host_prep(
        inputs["inputs"],
        inputs["H"],
        inputs["sigma2"],
        inputs["input_ponderation"],
        inputs["w_cv"],
        inputs["gate_logit"],
    )
    nc = _get_nc(gate)
    res = run_bass_kernel_spmd(
        nc, in_maps, core_ids=list(range(NCORES)), trace=trace
    )
    posts_raw = np.concatenate(
        [r["posts"].reshape(BL, T, NVAR) for r in res.results], axis=0
    )
    out = _host_post(
        posts_raw, xs, norm_llrs,
        inputs["out_ponderation"], inputs["skip_ponderation"],
    )
    return out, res


def kernel(**inputs) -> np.ndarray:
    out, _ = run(trace=False, **inputs)
    return out
